# revision 10
# baseline (speedup 1.0000x reference)
"""Causal self-attention with learned token pruning (LTP) on 8 Trainium2 cores.

Problem: B=4, T=2048, C=768, H=12 heads of 64.
  qkv = x @ w_attn; causal+mask softmax attention; y = att_out @ w_proj;
  importance = att.mean(heads).mean(queries).

Sharding: core = (batch b, head-group g) with g in {0,1} covering 6 heads.
Each core computes its batch's qkv for its 6 heads, full causal attention
in S^T layout (keys on partitions), the partial y = att_out @ w_proj rows
for its heads, and the partial importance (sum over its 6 heads).
Host sums the two partial y / importance per batch (no on-device collectives).

On-device layout notes:
 - All matmuls in bf16 with fp32 PSUM accumulation.
 - Scores computed transposed: S^T[k, q] = K_blk^T.T @ Q^T, head pairs packed
   into the 128-row PE array via tile_position (contract dim is 64).
 - exp via ACT with per-key mask bias (per-partition AP) and scale=1/8.
 - AV: lhsT = Vaug [128 keys, 65] (col 64 = ones -> Z row), rhs = E tiles.
 - Normalization: recipZ row broadcast over partitions via K=1 matmul, then
   one DVE multiply; importance via fused tensor_tensor_reduce of E * bcast.
"""
import os
import numpy as np
import ml_dtypes

import concourse.bass as bass
import concourse.mybir as mybir
from concourse import bacc
from concourse.tile import TileContext
from concourse.bass_utils import run_bass_kernel_spmd

B, T, C, H = 4, 2048, 768, 12
HS = 64
G = 2              # head groups (tensor-parallel factor)
HPG = H // G       # heads per group = 6
NF = HPG * HS      # per-core qkv feature width = 384
P = 128
NKB = T // P       # 16 key blocks
NSTRIP = 4         # query strips of 512
SW = T // NSTRIP   # strip width 512
CT = C // P        # 6 contraction tiles
FT = NF // P       # 3 feature tiles (= head pairs)
MASK_BIAS = -40.0

BF = mybir.dt.bfloat16
F32 = mybir.dt.float32

_CACHE = {}


def _build():
    if "nc" in _CACHE:
        return _CACHE["nc"]
    nc = bacc.Bacc("TRN2", target_bir_lowering=False, debug=False, num_devices=8)

    xt_d = nc.dram_tensor("xt", [P, CT, T], BF, kind="ExternalInput")
    wq_d = nc.dram_tensor("wq", [P, CT, NF], BF, kind="ExternalInput")
    wk_d = nc.dram_tensor("wk", [P, CT, NF], BF, kind="ExternalInput")
    wv_d = nc.dram_tensor("wv", [P, CT, NF], BF, kind="ExternalInput")
    wp_d = nc.dram_tensor("wp", [P, FT, C], BF, kind="ExternalInput")
    kb_d = nc.dram_tensor("kbias", [P, NKB], F32, kind="ExternalInput")

    y_d = nc.dram_tensor("y", [T, C], F32, kind="ExternalOutput")
    imp_d = nc.dram_tensor("imp", [P, NKB], F32, kind="ExternalOutput")

    with TileContext(nc) as tc:
        with tc.tile_pool(name="inp", bufs=1) as inp, \
             tc.tile_pool(name="qkv", bufs=1) as qkv, \
             tc.tile_pool(name="epool", bufs=36) as epool, \
             tc.tile_pool(name="small", bufs=2) as small, \
             tc.tile_pool(name="ystage", bufs=3) as ystage, \
             tc.tile_pool(name="ps", bufs=6, space="PSUM") as ps, \
             tc.tile_pool(name="avps", bufs=2, space="PSUM") as avps:

            xt = inp.tile([P, CT, T], BF)
            wq = inp.tile([P, CT, NF], BF)
            wk = inp.tile([P, CT, NF], BF)
            wv = inp.tile([P, CT, NF], BF)
            wp = inp.tile([P, FT, C], BF)
            kbias = inp.tile([P, NKB], F32)
            nc.sync.dma_start(xt[:], xt_d[:])
            nc.sync.dma_start(wq[:], wq_d[:])
            nc.sync.dma_start(wk[:], wk_d[:])
            nc.sync.dma_start(wv[:], wv_d[:])
            nc.sync.dma_start(wp[:], wp_d[:])
            nc.sync.dma_start(kbias[:], kb_d[:])

            ones1 = inp.tile([1, P], F32)
            nc.vector.memset(ones1[:], 1.0)
            imp_acc = inp.tile([P, NKB], F32)
            # importance slots: per key block kb, one accum column per
            # (head, strip) contribution; reduced at the end (the fused
            # accumulate-in-place chain hangs HW, so keep slots independent)
            slot_base = []
            off = 0
            for kb in range(NKB):
                slot_base.append(off)
                off += 2 * FT * (NSTRIP - kb // 4)
            accwide = inp.tile([P, off], F32)
            slot_ctr = [0] * NKB

            # ---- qkv phase ----
            qt = qkv.tile([P, FT, T], BF)      # Q^T [feat, tok]
            kt = qkv.tile([P, FT, T], BF)      # K^T
            vsb = qkv.tile([P, NKB, HPG, 65], BF)  # Vaug per key block / head
            attn = qkv.tile([P, FT, T], BF)    # normalized att_out^T

            nc.vector.memset(vsb[:, :, :, 64:65], 1.0)

            for ft in range(FT):
                for seg in range(NSTRIP):
                    for dst, w in ((qt, wq), (kt, wk)):
                        pst = ps.tile([P, SW], F32, tag="big")
                        for ct in range(CT):
                            nc.tensor.matmul(
                                pst[:],
                                w[:, ct, ft * P:(ft + 1) * P],
                                xt[:, ct, seg * SW:(seg + 1) * SW],
                                start=(ct == 0), stop=(ct == CT - 1))
                        nc.any.tensor_copy(
                            dst[:, ft, seg * SW:(seg + 1) * SW], pst[:])
            for m in range(NKB):
                pst = ps.tile([P, NF], F32, tag="big")
                for ct in range(CT):
                    nc.tensor.matmul(
                        pst[:],
                        xt[:, ct, m * P:(m + 1) * P],
                        wv[:, ct, :],
                        start=(ct == 0), stop=(ct == CT - 1))
                nc.any.tensor_copy(
                    vsb[:, m, :, 0:64],
                    pst[:].rearrange("p (h f) -> p h f", f=HS))

            # ---- attention per head-pair / strip ----
            for hp in range(FT):
                for s in range(NSTRIP):
                    q0 = s * SW
                    nkb = 4 * s + 4     # key blocks covering this strip
                    av = [avps.tile([65, SW], F32, tag="av", name=f"av{hp}_{s}_{i}")
                          for i in range(2)]
                    e_tiles = []
                    for kb in range(nkb):
                        dc = kb - 4 * s
                        col0 = max(0, dc * P)
                        sps = [ps.tile([P, SW], F32, tag="big",
                                       name=f"s{hp}_{s}_{kb}_{i}")
                               for i in range(2)]
                        nc.tensor.matmul(
                            sps[0][:, col0:],
                            kt[0:64, hp, kb * P:(kb + 1) * P],
                            qt[0:64, hp, q0 + col0:q0 + SW],
                            start=True, stop=True)
                        nc.tensor.matmul(
                            sps[1][:, col0:],
                            kt[64:128, hp, kb * P:(kb + 1) * P],
                            qt[64:128, hp, q0 + col0:q0 + SW],
                            start=True, stop=True, tile_position=(64, 0))
                        epair = []
                        for i in range(2):
                            e = epool.tile([P, SW], BF, tag="e",
                                           name=f"e{hp}_{s}_{kb}_{i}")
                            nc.scalar.activation(
                                e[:, col0:], sps[i][:, col0:],
                                mybir.ActivationFunctionType.Exp,
                                bias=kbias[:, kb:kb + 1], scale=0.125)
                            if dc >= 0:
                                # keep k<=q  <=>  (q - k) >= 0
                                nc.gpsimd.affine_select(
                                    out=e[:, col0:col0 + P],
                                    in_=e[:, col0:col0 + P],
                                    compare_op=mybir.AluOpType.is_ge,
                                    fill=0.0, base=0,
                                    pattern=[[1, P]], channel_multiplier=-1)
                            h = 2 * hp + i
                            nc.tensor.matmul(
                                av[i][:, col0:],
                                vsb[:, kb, h, :],
                                e[:, col0:],
                                start=(kb == 0), stop=(kb == nkb - 1))
                            epair.append(e)
                        e_tiles.append(epair)
                    for i in range(2):
                        h = 2 * hp + i
                        rz = small.tile([1, SW], F32, tag="rz")
                        nc.vector.reciprocal(rz[:], av[i][64:65, :])
                        bc_ps = ps.tile([P, SW], F32, tag="big",
                                        name=f"bc{hp}_{s}_{i}")
                        nc.tensor.matmul(bc_ps[:], ones1[:], rz[:],
                                         start=True, stop=True)
                        bc = small.tile([P, SW], F32, tag="bc")
                        nc.any.tensor_copy(bc[:], bc_ps[:])
                        nc.vector.tensor_tensor(
                            attn[64 * i:64 * i + 64, hp, q0:q0 + SW],
                            av[i][0:64, :], bc[0:64, :],
                            mybir.AluOpType.mult)
                        for kb in range(nkb):
                            col0 = max(0, (kb - 4 * s) * P)
                            scratch = small.tile([P, SW], BF, tag="scratch")
                            slot = slot_base[kb] + slot_ctr[kb]
                            slot_ctr[kb] += 1
                            nc.vector.scalar_tensor_tensor(
                                out=scratch[:, col0:],
                                in0=e_tiles[kb][i][:, col0:],
                                scalar=1.0,
                                in1=bc[:, col0:],
                                op0=mybir.AluOpType.mult,
                                op1=mybir.AluOpType.mult,
                                accum_out=accwide[:, slot:slot + 1])

            # ---- proj phase ----
            for m in range(NKB):
                halves = []
                for hf in range(2):
                    pst = ps.tile([P, C // 2], F32, tag="big",
                                  name=f"proj{m}_{hf}")
                    for ft in range(FT):
                        nc.tensor.matmul(
                            pst[:],
                            attn[:, ft, m * P:(m + 1) * P],
                            wp[:, ft, hf * (C // 2):(hf + 1) * (C // 2)],
                            start=(ft == 0), stop=(ft == FT - 1))
                    halves.append(pst)
                yst = ystage.tile([P, C], F32, tag="y")
                for hf in range(2):
                    nc.any.tensor_copy(
                        yst[:, hf * (C // 2):(hf + 1) * (C // 2)], halves[hf][:])
                nc.sync.dma_start(y_d[m * P:(m + 1) * P, :], yst[:])

            for kb in range(NKB):
                cnt = 2 * FT * (NSTRIP - kb // 4)
                nc.vector.tensor_reduce(
                    imp_acc[:, kb:kb + 1],
                    accwide[:, slot_base[kb]:slot_base[kb] + cnt],
                    axis=mybir.AxisListType.X, op=mybir.AluOpType.add)
            nc.sync.dma_start(imp_d[:], imp_acc[:])

    nc.compile()
    _CACHE["nc"] = nc
    return nc


def _prep_core(x, mask, w_attn, w_proj, b, g):
    bf = ml_dtypes.bfloat16

    def tile3(a, n0, n1, n2):
        # [n0*n1, n2] -> [n1, n0, n2] with partition dim first
        return np.ascontiguousarray(
            a.reshape(n0, n1, n2).transpose(1, 0, 2)).astype(bf)

    xt = np.ascontiguousarray(x[b].T)                     # [C, T]
    cols = slice(g * NF, (g + 1) * NF)
    kcols = slice(C + g * NF, C + (g + 1) * NF)
    vcols = slice(2 * C + g * NF, 2 * C + (g + 1) * NF)
    # bias = 0 for valid keys, MASK_BIAS (-40) for masked keys
    kbias = ((mask[b].astype(np.float32) - 1.0) * -MASK_BIAS).reshape(NKB, P).T
    return {
        "xt": tile3(xt, CT, P, T),
        "wq": tile3(w_attn[:, cols], CT, P, NF),
        "wk": tile3(w_attn[:, kcols], CT, P, NF),
        "wv": tile3(w_attn[:, vcols], CT, P, NF),
        "wp": tile3(w_proj[g * NF:(g + 1) * NF, :], FT, P, C),
        "kbias": np.ascontiguousarray(kbias).astype(np.float32),
    }


def kernel(x, attention_mask, w_attn, w_proj):
    x = np.asarray(x, dtype=np.float32)
    mask = np.asarray(attention_mask, dtype=np.int32)
    w_attn = np.asarray(w_attn, dtype=np.float32)
    w_proj = np.asarray(w_proj, dtype=np.float32)

    nc = _build()
    in_maps = []
    for core in range(8):
        b, g = core // G, core % G
        in_maps.append(_prep_core(x, mask, w_attn, w_proj, b, g))

    trace = bool(int(os.environ.get("KERNEL_TRACE", "0")))
    res = run_bass_kernel_spmd(nc, in_maps, core_ids=list(range(8)),
                               trace=trace)
    _CACHE["last_result"] = res

    y = np.zeros((B, T, C), dtype=np.float32)
    imp = np.zeros((B, T), dtype=np.float32)
    for core in range(8):
        b = core // G
        r = res.results[core]
        y[b] += r["y"]
        imp[b] += r["imp"].T.reshape(T)
    imp /= (H * T)
    return y, imp


# revision 17
# speedup vs baseline: 1.1903x; 1.1903x over previous
"""Causal self-attention with learned token pruning (LTP) on 8 Trainium2 cores.

Problem: B=4, T=2048, C=768, H=12 heads of 64.
  qkv = x @ w_attn; causal+mask softmax attention; y = att_out @ w_proj;
  importance = att.mean(heads).mean(queries).

Sharding: core = (batch b, head-group g) with g in {0,1} covering 6 heads.
Each core computes its batch's qkv for its 6 heads, full causal attention
in S^T layout (keys on partitions), the partial y = att_out @ w_proj rows
for its heads, and the partial importance (sum over its 6 heads).
Host sums the two partial y / importance per batch (no on-device collectives).

On-device layout notes:
 - All matmuls in bf16 with fp32 PSUM accumulation.
 - Scores computed transposed: S^T[k, q] = K_blk^T.T @ Q^T, head pairs packed
   into the 128-row PE array via tile_position (contract dim is 64).
 - exp via ACT with per-key mask bias (per-partition AP) and scale=1/8.
 - AV: lhsT = Vaug [128 keys, 65] (col 64 = ones -> Z row), rhs = E tiles.
 - Normalization: recipZ row broadcast over partitions via K=1 matmul, then
   one DVE multiply; importance via fused tensor_tensor_reduce of E * bcast.
"""
import os
import numpy as np
import ml_dtypes

import concourse.bass as bass
import concourse.mybir as mybir
from concourse import bacc
from concourse.tile import TileContext
from concourse.bass_utils import run_bass_kernel_spmd

B, T, C, H = 4, 2048, 768, 12
HS = 64
G = 2              # head groups (tensor-parallel factor)
HPG = H // G       # heads per group = 6
NF = HPG * HS      # per-core qkv feature width = 384
P = 128
NKB = T // P       # 16 key blocks
NSTRIP = 4         # query strips of 512
SW = T // NSTRIP   # strip width 512
CT = C // P        # 6 contraction tiles
FT = NF // P       # 3 feature tiles (= head pairs)
MASK_BIAS = -40.0

BF = mybir.dt.bfloat16
F32 = mybir.dt.float32

_CACHE = {}


def _build():
    if "nc" in _CACHE:
        return _CACHE["nc"]
    nc = bacc.Bacc("TRN2", target_bir_lowering=False, debug=False, num_devices=8)

    xt_d = nc.dram_tensor("xt", [P, CT, T], BF, kind="ExternalInput")
    wq_d = nc.dram_tensor("wq", [P, CT, NF], BF, kind="ExternalInput")
    wk_d = nc.dram_tensor("wk", [P, CT, NF], BF, kind="ExternalInput")
    wv_d = nc.dram_tensor("wv", [P, CT, NF], BF, kind="ExternalInput")
    wp_d = nc.dram_tensor("wp", [P, FT, C], BF, kind="ExternalInput")
    kb_d = nc.dram_tensor("kbias", [P, NKB], F32, kind="ExternalInput")

    y_d = nc.dram_tensor("y", [T, C], F32, kind="ExternalOutput")
    imp_d = nc.dram_tensor("imp", [P, NKB], F32, kind="ExternalOutput")

    with TileContext(nc) as tc:
        with tc.tile_pool(name="inp", bufs=1) as inp, \
             tc.tile_pool(name="qkv", bufs=1) as qkv, \
             tc.tile_pool(name="epool", bufs=36) as epool, \
             tc.tile_pool(name="small", bufs=2) as small, \
             tc.tile_pool(name="ystage", bufs=3) as ystage, \
             tc.tile_pool(name="ps", bufs=6, space="PSUM") as ps, \
             tc.tile_pool(name="avps", bufs=2, space="PSUM") as avps:

            xt = inp.tile([P, CT, T], BF)
            wq = inp.tile([P, CT, NF], BF)
            wk = inp.tile([P, CT, NF], BF)
            wv = inp.tile([P, CT, NF], BF)
            wp = inp.tile([P, FT, C], BF)
            kbias = inp.tile([P, NKB], F32)
            nc.sync.dma_start(xt[:], xt_d[:])
            nc.sync.dma_start(wq[:], wq_d[:])
            nc.sync.dma_start(wk[:], wk_d[:])
            nc.sync.dma_start(wv[:], wv_d[:])
            nc.sync.dma_start(wp[:], wp_d[:])
            nc.sync.dma_start(kbias[:], kb_d[:])

            ones1 = inp.tile([33, P], F32)
            nc.vector.memset(ones1[:], 1.0)
            imp_acc = inp.tile([P, NKB], F32)
            # importance slots: per key block kb, one accum column per
            # (head, strip) contribution; reduced at the end (the fused
            # accumulate-in-place chain hangs HW, so keep slots independent)
            slot_base = []
            off = 0
            for kb in range(NKB):
                slot_base.append(off)
                off += 2 * FT * (NSTRIP - kb // 4)
            accwide = inp.tile([P, off], F32)
            slot_ctr = [0] * NKB

            # ---- qkv phase ----
            qt = qkv.tile([P, FT, T], BF)      # Q^T [feat, tok]
            kt = qkv.tile([P, FT, T], BF)      # K^T
            vsb = qkv.tile([P, NKB, HPG, 65], BF)  # Vaug per key block / head
            attn = qkv.tile([P, FT, T], BF)    # normalized att_out^T

            nc.vector.memset(vsb[:, :, :, 64:65], 1.0)

            for ft in range(FT):
                for seg in range(NSTRIP):
                    for dst, w in ((qt, wq), (kt, wk)):
                        pst = ps.tile([P, SW], F32, tag="big")
                        for ct in range(CT):
                            nc.tensor.matmul(
                                pst[:],
                                w[:, ct, ft * P:(ft + 1) * P],
                                xt[:, ct, seg * SW:(seg + 1) * SW],
                                start=(ct == 0), stop=(ct == CT - 1))
                        nc.scalar.copy(
                            dst[:, ft, seg * SW:(seg + 1) * SW], pst[:])
            for m in range(NKB):
                pst = ps.tile([P, NF], F32, tag="big")
                for ct in range(CT):
                    nc.tensor.matmul(
                        pst[:],
                        xt[:, ct, m * P:(m + 1) * P],
                        wv[:, ct, :],
                        start=(ct == 0), stop=(ct == CT - 1))
                nc.vector.tensor_copy(
                    vsb[:, m, :, 0:64],
                    pst[:].rearrange("p (h f) -> p h f", f=HS))

            # ---- attention per head-pair / strip ----
            for hp in range(FT):
                for s in range(NSTRIP):
                    q0 = s * SW
                    nkb = 4 * s + 4     # key blocks covering this strip
                    av = [avps.tile([65, SW], F32, tag="av", name=f"av{hp}_{s}_{i}")
                          for i in range(2)]
                    e_tiles = []
                    for kb in range(nkb):
                        dc = kb - 4 * s
                        col0 = max(0, dc * P)
                        sps = [ps.tile([P, SW], F32, tag="big",
                                       name=f"s{hp}_{s}_{kb}_{i}")
                               for i in range(2)]
                        nc.tensor.matmul(
                            sps[0][:, col0:],
                            kt[0:64, hp, kb * P:(kb + 1) * P],
                            qt[0:64, hp, q0 + col0:q0 + SW],
                            start=True, stop=True)
                        nc.tensor.matmul(
                            sps[1][:, col0:],
                            kt[64:128, hp, kb * P:(kb + 1) * P],
                            qt[64:128, hp, q0 + col0:q0 + SW],
                            start=True, stop=True, tile_position=(64, 0))
                        epair = []
                        for i in range(2):
                            e = epool.tile([P, SW], BF, tag="e",
                                           name=f"e{hp}_{s}_{kb}_{i}")
                            nc.scalar.activation(
                                e[:, col0:], sps[i][:, col0:],
                                mybir.ActivationFunctionType.Exp,
                                bias=kbias[:, kb:kb + 1], scale=0.125)
                            if dc >= 0:
                                # keep k<=q  <=>  (q - k) >= 0
                                nc.gpsimd.affine_select(
                                    out=e[:, col0:col0 + P],
                                    in_=e[:, col0:col0 + P],
                                    compare_op=mybir.AluOpType.is_ge,
                                    fill=0.0, base=0,
                                    pattern=[[1, P]], channel_multiplier=-1)
                            h = 2 * hp + i
                            nc.tensor.matmul(
                                av[i][:, col0:],
                                vsb[:, kb, h, :],
                                e[:, col0:],
                                start=(kb == 0), stop=(kb == nkb - 1))
                            epair.append(e)
                        e_tiles.append(epair)
                    # batch the two heads' Z rows into one reciprocal: rows
                    # 0 and 32 of a [33, SW] tile (32 so matmul rhs
                    # base_partition stays legal)
                    zt = small.tile([33, SW], F32, tag="zt")
                    nc.vector.memset(zt[:], 1.0)
                    nc.vector.tensor_copy(zt[0:1, :], av[0][64:65, :])
                    nc.vector.tensor_copy(zt[32:33, :], av[1][64:65, :])
                    rzt = small.tile([33, SW], F32, tag="rzt")
                    nc.vector.reciprocal(rzt[:], zt[:])
                    for i in range(2):
                        h = 2 * hp + i
                        bc_ps = ps.tile([P, SW], F32, tag="big",
                                        name=f"bc{hp}_{s}_{i}")
                        nc.tensor.matmul(bc_ps[:],
                                         ones1[32 * i:32 * i + 1, :],
                                         rzt[32 * i:32 * i + 1, :],
                                         start=True, stop=True)
                        bc = small.tile([P, SW], BF, tag="bc")
                        nc.vector.tensor_copy(bc[:], bc_ps[:])
                        nc.vector.tensor_tensor(
                            attn[64 * i:64 * i + 64, hp, q0:q0 + SW],
                            av[i][0:64, :], bc[0:64, :],
                            mybir.AluOpType.mult)
                        for kb in range(nkb):
                            col0 = max(0, (kb - 4 * s) * P)
                            scratch = small.tile([P, SW], BF, tag="scratch")
                            slot = slot_base[kb] + slot_ctr[kb]
                            slot_ctr[kb] += 1
                            nc.vector.scalar_tensor_tensor(
                                out=scratch[:, col0:],
                                in0=e_tiles[kb][i][:, col0:],
                                scalar=1.0,
                                in1=bc[:, col0:],
                                op0=mybir.AluOpType.mult,
                                op1=mybir.AluOpType.mult,
                                accum_out=accwide[:, slot:slot + 1])

            # ---- proj phase ----
            for m in range(NKB):
                halves = []
                for hf in range(2):
                    pst = ps.tile([P, C // 2], F32, tag="big",
                                  name=f"proj{m}_{hf}")
                    for ft in range(FT):
                        nc.tensor.matmul(
                            pst[:],
                            attn[:, ft, m * P:(m + 1) * P],
                            wp[:, ft, hf * (C // 2):(hf + 1) * (C // 2)],
                            start=(ft == 0), stop=(ft == FT - 1))
                    halves.append(pst)
                yst = ystage.tile([P, C], F32, tag="y")
                for hf in range(2):
                    nc.scalar.copy(
                        yst[:, hf * (C // 2):(hf + 1) * (C // 2)], halves[hf][:])
                nc.sync.dma_start(y_d[m * P:(m + 1) * P, :], yst[:])

            for kb in range(NKB):
                cnt = 2 * FT * (NSTRIP - kb // 4)
                nc.vector.tensor_reduce(
                    imp_acc[:, kb:kb + 1],
                    accwide[:, slot_base[kb]:slot_base[kb] + cnt],
                    axis=mybir.AxisListType.X, op=mybir.AluOpType.add)
            nc.sync.dma_start(imp_d[:], imp_acc[:])

    nc.compile()
    _CACHE["nc"] = nc
    return nc


def _prep_core(x, mask, w_attn, w_proj, b, g):
    bf = ml_dtypes.bfloat16

    def tile3(a, n0, n1, n2):
        # [n0*n1, n2] -> [n1, n0, n2] with partition dim first
        return np.ascontiguousarray(
            a.reshape(n0, n1, n2).transpose(1, 0, 2)).astype(bf)

    xt = np.ascontiguousarray(x[b].T)                     # [C, T]
    cols = slice(g * NF, (g + 1) * NF)
    kcols = slice(C + g * NF, C + (g + 1) * NF)
    vcols = slice(2 * C + g * NF, 2 * C + (g + 1) * NF)
    # bias = 0 for valid keys, MASK_BIAS (-40) for masked keys
    kbias = ((mask[b].astype(np.float32) - 1.0) * -MASK_BIAS).reshape(NKB, P).T
    return {
        "xt": tile3(xt, CT, P, T),
        "wq": tile3(w_attn[:, cols], CT, P, NF),
        "wk": tile3(w_attn[:, kcols], CT, P, NF),
        "wv": tile3(w_attn[:, vcols], CT, P, NF),
        "wp": tile3(w_proj[g * NF:(g + 1) * NF, :], FT, P, C),
        "kbias": np.ascontiguousarray(kbias).astype(np.float32),
    }


def kernel(x, attention_mask, w_attn, w_proj):
    x = np.asarray(x, dtype=np.float32)
    mask = np.asarray(attention_mask, dtype=np.int32)
    w_attn = np.asarray(w_attn, dtype=np.float32)
    w_proj = np.asarray(w_proj, dtype=np.float32)

    nc = _build()
    in_maps = []
    for core in range(8):
        b, g = core // G, core % G
        in_maps.append(_prep_core(x, mask, w_attn, w_proj, b, g))

    trace = bool(int(os.environ.get("KERNEL_TRACE", "0")))
    res = run_bass_kernel_spmd(nc, in_maps, core_ids=list(range(8)),
                               trace=trace)
    _CACHE["last_result"] = res

    y = np.zeros((B, T, C), dtype=np.float32)
    imp = np.zeros((B, T), dtype=np.float32)
    for core in range(8):
        b = core // G
        r = res.results[core]
        y[b] += r["y"]
        imp[b] += r["imp"].T.reshape(T)
    imp /= (H * T)
    return y, imp
